# revision 22
# baseline (speedup 1.0000x reference)
"""Channel-attention kernel for Trainium2, data-parallel over batch on 8 NeuronCores.

Reference computation (per batch b):
    xr   = x[b].reshape(HW, C)                  # [4096, 512] fp32
    s    = xr^T @ xr                            # [C, C] gram matrix
    attn = softmax(s, axis=-1)
    v    = xr @ attn                            # [4096, 512]
    out  = beta * v + x[b]

Device strategy (per core: 2 batches, software-pipelined), v5:
  - the host shard step pre-packs three input views, each per-partition
    contiguous in DRAM so every load is a single-segment (cheap-trigger)
    DMA -- multi-segment triggers cost up to 14us of engine time:
      xb  = bf16 natural, half-batch packed   (epilogue, 2 loads/batch)\n    and the output is stored bf16 (the host upcasts to fp32): per-core DMA\n    sustains only ~300GB/s aggregate, so bytes are the wall -- bf16 stores\n    halve them at the same ~3e-4-per-element accuracy as the bf16 x load.
      xn  = fp8 natural, K-step packed        (GEMM1 operands, 4 loads/batch)
      xt  = fp8 x^T, channel-pair packed      (GEMM2 stationary, 2 loads/batch)
  - DMA-completion semaphore lanes are shared per tile-pool and waits use
    conservative emission-time thresholds, so each batch's input tiles live
    in their own pools (a later batch's in-flight loads must never gate an
    earlier batch's consumers).
  - both GEMMs run fp8 with perf_mode=DoubleRow (virtual 128x256 PE array).
    GEMM1 runs as two half-passes (cb{0,1} then cb{2,3}) so only 2 PSUM
    banks hold gram tiles and the first half's softmax hides under the
    second half's matmuls; each half ends cb-major for the same reason.
  - softmax rows on DVE+ScalarE out of PSUM; beta folded into the
    normalization (attn_scaled = beta * exp(s-max) / sum, written fp8), so
    beta=0 gives v==0 and out = fp32(bf16(x)) exactly.
  - GEMM2 into a 3-deep [128,1024] PSUM ring; the first chunks issue their
    channel-pair-0 matmuls only so the PE restarts before the last softmax
    finishes.  Epilogue (out = v + x): two of three chunks DVE-add straight
    from PSUM, every third is ScalarE-evicted to SBUF and GpSimd-added
    (GpSimd cannot read PSUM); 256-row chunks pair into 512-row stores
    alternating the sync/gpsimd queues.
"""

import ml_dtypes
import numpy as np

import concourse.bass as bass
import concourse.tile as tile
from concourse import bacc, mybir
from concourse.bass_utils import run_bass_kernel_spmd

N_CORES = 8
B_FULL = 16
B_PER_CORE = B_FULL // N_CORES  # 2
H = 64
W = 64
HW = H * W  # 4096
C = 512
NT = HW // 128  # 32 row tiles
CB = C // 128  # 4 channel blocks
NK = NT // 2  # 16 DoubleRow K-steps (256 rows each)
NKC = 4  # xn load chunks per batch (4 K-steps each)
NCH = NT // 2  # 16 GEMM2/epilogue chunks (256 rows each)
NSC = NT // 4  # 8 store super-chunks (512 rows each)
TAILK = 4  # K-steps per half-pass that run cb-major (softmax overlap)
# store spans (sc, ch0, ch1): big early stores amortize per-DMA cost, small
# final ones shorten the drain tail
STORE_SPANS = [(0, 0, 4), (1, 4, 8), (2, 8, 12), (3, 12, 14), (4, 14, 16)]
PREFIX = 3  # GEMM2 chunks that issue pair-0 matmuls before pair-1 is ready

F32 = mybir.dt.float32
BF16 = mybir.dt.bfloat16
FP8 = mybir.dt.float8e4
AXL = mybir.AxisListType
ALU = mybir.AluOpType
ACTFN = mybir.ActivationFunctionType
DR = mybir.MatmulPerfMode.DoubleRow


class BatchState:
    def __init__(self):
        self.xbf = {}  # eighth -> (tile, base_eighth) bf16 natural layout
        self.x8 = {}  # k-step -> [128, 2C] fp8 cast tiles (natural layout)
        self.xt8 = {}  # pair -> [128, 2*HW] fp8 tile (x^T, paired channels)
        self.s_ps = {}  # cb -> [128, C] f32 PSUM
        self.ot = {}  # store-span -> bf16 staging tile
        self.attn = {}  # pair -> [128, 2C] fp8 tile


def emit_xb_span(nc, pools, xb_ap, b, e0, e1, st, eng):
    """One DMA covering eighths [e0, e1) of batch b's bf16 natural view."""
    n = e1 - e0
    t = pools[f"xbf_{b}_{e0}"].tile(
        [128, n * 4 * C], BF16, tag="xbf", name=f"xbf_b{b}_e{e0}"
    )
    eng.dma_start(
        t[:, :].rearrange("p (e w) -> p e w", e=n),
        xb_ap[b, e0:e1, :, :].rearrange("e p w -> p e w"),
    )
    for e in range(e0, e1):
        st.xbf[e] = (t, e0)


def emit_all_loads(nc, pools, aps, states):
    """All loads up-front: aggregate DMA runs at its ~420GB/s cap, so what
    matters is per-queue deadline order and >=1MB DMAs (per-DMA queue cost
    ~2-3us).  Batch 0's bf16 view interleaves both queues in 1MB pairs so
    its casts are fed from ~12us; everything else follows by deadline."""
    xb_ap, xt_ap = aps
    emit_xb_span(nc, pools, xb_ap, 0, 0, 1, states[0], nc.sync)
    emit_xb_span(nc, pools, xb_ap, 0, 1, 2, states[0], nc.scalar)
    emit_xb_span(nc, pools, xb_ap, 0, 2, 4, states[0], nc.sync)
    emit_xb_span(nc, pools, xb_ap, 0, 4, 6, states[0], nc.scalar)
    emit_xb_span(nc, pools, xb_ap, 0, 6, 8, states[0], nc.sync)
    xt0 = pools["xt8_0"].tile([128, 4 * HW], FP8, tag="xt8", name="xt8_b0")
    nc.scalar.dma_start(
        xt0[:, :].rearrange("p (q w) -> p q w", q=2),
        xt_ap[0, :, :, :].rearrange("q p w -> p q w"),
    )
    states[0].xt8[0] = xt0
    emit_xb_span(nc, pools, xb_ap, 1, 0, 4, states[1], nc.sync)
    emit_xb_span(nc, pools, xb_ap, 1, 4, 8, states[1], nc.scalar)
    xt1 = pools["xt8_1"].tile([128, 4 * HW], FP8, tag="xt8", name="xt8_b1")
    nc.sync.dma_start(
        xt1[:, :].rearrange("p (q w) -> p q w", q=2),
        xt_ap[1, :, :, :].rearrange("q p w -> p q w"),
    )
    states[1].xt8[0] = xt1


def emit_cast(nc, pools, b, k, st, eng):
    """fp8 shadow of K-step k (row tiles 2k, 2k+1) on ScalarE or DVE."""
    e = (2 * k) // 4
    tile_, base = st.xbf[e]
    off = 2 * k - base * 4
    t = pools["x8"].tile([128, 2 * C], FP8, tag="x8", name=f"x8_b{b}_k{k}")
    src_slice = tile_[:, off * C : (off + 2) * C]
    if eng == "v":
        nc.vector.tensor_copy(t[:, :], src_slice)
    elif eng == "g":
        nc.gpsimd.tensor_copy(t[:, :], src_slice)
    else:
        nc.scalar.copy(t[:, :], src_slice)
    st.x8[k] = t


def emit_g1_step(nc, pools, b, k, cb, st):
    if k == 0:
        st.s_ps[cb] = pools["ps_s"].tile([128, C], F32, tag="s", name=f"s_b{b}_{cb}")
    r3 = st.x8[k][:, :].rearrange("p (o c) -> p o c", o=2)
    nc.tensor.matmul(
        st.s_ps[cb][:, :],
        r3[:, :, cb * 128 : (cb + 1) * 128],
        r3,
        start=(k == 0),
        stop=(k == NK - 1),
        perf_mode=DR,
    )


def emit_softmax(nc, pools, beta_bc, b, cb, st):
    """softmax row block cb out of PSUM -> fp8 half of the attn pair tile."""
    pair, o = cb // 2, cb % 2
    nmax = pools["st"].tile([128, 1], F32, tag="nmax")
    nc.vector.tensor_reduce(
        nmax[:, :], st.s_ps[cb][:, :], axis=AXL.X, op=ALU.max, negate=True
    )
    exps = pools["sm"].tile([128, C], BF16, tag="exps")
    ssum = pools["st"].tile([128, 1], F32, tag="ssum")
    nc.scalar.activation(
        exps[:, :],
        st.s_ps[cb][:, :],
        ACTFN.Exp,
        bias=nmax[:, :],
        scale=1.0,
        accum_out=ssum[:, :],
    )
    rinv = pools["st"].tile([128, 1], F32, tag="rinv")
    nc.vector.reciprocal(rinv[:, :], ssum[:, :])
    rsc = pools["st"].tile([128, 1], F32, tag="rsc")
    nc.vector.tensor_mul(rsc[:, :], rinv[:, :], beta_bc[:, :])
    if o == 0:
        at = pools["attn"].tile(
            [128, 2 * C], FP8, tag="attn", name=f"attn_b{b}_p{pair}"
        )
        st.attn[pair] = at
    nc.scalar.activation(
        st.attn[pair][:, o * C : (o + 1) * C], exps[:, :], ACTFN.Copy, scale=rsc[:, :]
    )


def emit_g1_half(
    nc, pools, beta_bc, b, half, st, cast_self=False, cast_next=None, after_k1=None
):
    """Half-pass over cb pair (2*half, 2*half+1); ends cb-major + softmax.
    cast_self: emit this batch's own fp8 casts alongside (pass A of batch 0).
    cast_next: BatchState of the next batch whose casts hide in this pass.
    after_k1: emitted once after K-step 1 (bulk load triggers go here so the
    first casts/matmuls aren't queued behind them)."""
    cbs = (2 * half, 2 * half + 1)
    for k in range(NK - TAILK):
        if cast_self:
            emit_cast(nc, pools, b, k, st, "s" if k % 2 == 0 else "v")
        for cb in cbs:
            emit_g1_step(nc, pools, b, k, cb, st)
        if k == 1 and after_k1 is not None:
            after_k1()
        if cast_next is not None:
            with nc._tile_ctx.tile_wait_until(0.035):
                emit_cast(nc, pools, b + 1, k, cast_next, "s" if k % 3 == 0 else "v")
    for k in range(NK - TAILK, NK):
        if cast_self:
            emit_cast(nc, pools, b, k, st, "s" if k % 2 == 0 else "v")
        if cast_next is not None:
            with nc._tile_ctx.tile_wait_until(0.035):
                emit_cast(nc, pools, b + 1, k, cast_next, "s" if k % 3 == 0 else "v")
    for cb in cbs:
        for k in range(NK - TAILK, NK):
            emit_g1_step(nc, pools, b, k, cb, st)
        emit_softmax(nc, pools, beta_bc, b, cb, st)


def emit_g2_mm(nc, b, ch, j, pair, st, vps):
    nt = ch * 2 + j
    xt3 = st.xt8[0][:, 2 * pair * HW : 2 * (pair + 1) * HW].rearrange(
        "p (o n) -> p o n", o=2
    )
    at3 = st.attn[pair][:, :].rearrange("p (o d) -> p o d", o=2)
    nc.tensor.matmul(
        vps[:, j * C : (j + 1) * C],
        xt3[:, :, nt * 128 : (nt + 1) * 128],
        at3,
        start=(pair == 0),
        stop=(pair == 1),
        perf_mode=DR,
    )


def emit_epilogue(nc, pools, out_ap, b, ch, st, vps):
    """out = v + x for chunk ch (row tiles 2ch, 2ch+1), then a 512KB store.
    Every fourth chunk takes the ScalarE-evict + GpSimd-add path to keep
    DVE off the critical path; stores alternate the sync/gpsimd queues."""
    sc, c0, c1 = next(x for x in STORE_SPANS if x[1] <= ch < x[2])
    if ch == c0:
        st.ot[sc] = pools["outp"].tile(
            [128, (c1 - c0) * 2 * C], BF16, tag=f"o{c1 - c0}",
            name=f"o_b{b}_s{sc}", bufs=3 if c1 - c0 == 4 else 2,
        )
    ot = st.ot[sc]
    oslice = ot[:, (ch - c0) * 2 * C : (ch - c0 + 1) * 2 * C]
    e = (2 * ch) // 4
    tile_, base = st.xbf[e]
    off = 2 * ch - base * 4
    xslice = tile_[:, off * C : (off + 2) * C]
    if ch % 4 == 1:
        tmp = pools["tmp"].tile([128, 2 * C], F32, tag="tmp")
        nc.scalar.copy(tmp[:, :], vps[:, :])
        nc.gpsimd.tensor_add(oslice, tmp[:, :], xslice)
    else:
        nc.vector.tensor_add(oslice, vps[:, :], xslice)
    if ch == c1 - 1:
        eng = nc.sync if sc % 2 == 0 else nc.scalar
        eng.dma_start(
            out_ap[b, c0 * 256 : c1 * 256, :].rearrange("(f p) c -> p f c", p=128),
            ot[:, :].rearrange("p (f c) -> p f c", c=C),
        )


def emit_g2(nc, pools, out_ap, b, st):
    vps_ring = {}
    for ch in range(PREFIX):
        vps_ring[ch] = pools["ps_v"].tile(
            [128, 2 * C], F32, tag="v", name=f"v_b{b}_c{ch}"
        )
        for j in range(2):
            emit_g2_mm(nc, b, ch, j, 0, st, vps_ring[ch])
    for ch in range(PREFIX):
        for j in range(2):
            emit_g2_mm(nc, b, ch, j, 1, st, vps_ring[ch])
        emit_epilogue(nc, pools, out_ap, b, ch, st, vps_ring[ch])
    for ch in range(PREFIX, NCH):
        vps = pools["ps_v"].tile([128, 2 * C], F32, tag="v", name=f"v_b{b}_c{ch}")
        for pair in range(2):
            for j in range(2):
                emit_g2_mm(nc, b, ch, j, pair, st, vps)
        emit_epilogue(nc, pools, out_ap, b, ch, st, vps)


def channel_attention_body(tc, out_ap, xb_ap, xt_ap, beta_ap):
    nc = tc.nc
    nc._tile_ctx = tc
    from contextlib import ExitStack

    with ExitStack() as ctx:
        ep = ctx.enter_context
        pools = {
            "attn": ep(tc.tile_pool(name="attn", bufs=4)),
            "sm": ep(tc.tile_pool(name="sm", bufs=3)),
            "st": ep(tc.tile_pool(name="st", bufs=8)),
            "outp": ep(tc.tile_pool(name="outp", bufs=3)),
            "tmp": ep(tc.tile_pool(name="tmp", bufs=3)),
            "const": ep(tc.tile_pool(name="const", bufs=1)),
            "ps_s": ep(tc.tile_pool(name="ps_s", bufs=2, space="PSUM")),
            "ps_v": ep(tc.tile_pool(name="ps_v", bufs=3, space="PSUM")),
        }
        pools["x8"] = ep(tc.tile_pool(name="x8", bufs=32))
        for b in range(B_PER_CORE):
            for e in (0, 1, 2, 4, 6) if b == 0 else (0, 4):
                pools[f"xbf_{b}_{e}"] = ep(tc.tile_pool(name=f"xbf_{b}_{e}", bufs=1))
        pools["xt8_0"] = ep(tc.tile_pool(name="xt8_0", bufs=1))
        pools["xt8_1"] = ep(tc.tile_pool(name="xt8_1", bufs=1))

        # beta -> broadcast to [128, 1] (gpsimd queue: keep sync/scalar clean)
        beta_sb = pools["const"].tile([1, 1], F32, tag="beta")
        nc.gpsimd.dma_start(beta_sb[0:1, 0:1], beta_ap[None, :])
        beta_bc = pools["const"].tile([128, 1], F32, tag="beta_bc")
        nc.gpsimd.partition_broadcast(beta_bc[:, :], beta_sb[0:1, :])

        aps = (xb_ap, xt_ap)
        states = [BatchState() for _ in range(B_PER_CORE)]
        emit_all_loads(nc, pools, aps, states)
        for b in range(B_PER_CORE):
            st = states[b]
            nxt = states[b + 1] if b + 1 < B_PER_CORE else None
            emit_g1_half(nc, pools, beta_bc, b, 0, st, cast_self=(b == 0))
            emit_g1_half(nc, pools, beta_bc, b, 1, st, cast_next=nxt)
            emit_g2(nc, pools, out_ap, b, st)


_NC_CACHE = None


def _build():
    global _NC_CACHE
    if _NC_CACHE is not None:
        return _NC_CACHE
    nc = bacc.Bacc(
        "TRN2",
        target_bir_lowering=False,
        debug=False,
        num_devices=N_CORES,
    )
    xb_ap = nc.dram_tensor(
        "xb", [B_PER_CORE, 8, 128, 4 * C], BF16, kind="ExternalInput"
    ).ap()
    xt_ap = nc.dram_tensor(
        "xt", [B_PER_CORE, 2, 128, 2 * HW], FP8, kind="ExternalInput"
    ).ap()
    beta_ap = nc.dram_tensor("beta", [1], F32, kind="ExternalInput").ap()
    out_ap = nc.dram_tensor(
        "out", [B_PER_CORE, HW, C], BF16, kind="ExternalOutput"
    ).ap()
    with tile.TileContext(nc) as tc:
        channel_attention_body(tc, out_ap, xb_ap, xt_ap, beta_ap)
    nc.compile()
    _NC_CACHE = nc
    return nc


def _prep_shard(xr, i):
    """Host-side input prep for core i: every view packed so each DMA is
    per-partition contiguous in DRAM."""
    xs = xr[i * B_PER_CORE : (i + 1) * B_PER_CORE]  # [2, HW, C] fp32
    # xb[b, q, p, f*C+c] = x[b, q*512 + f*128 + p, c]
    xb = np.ascontiguousarray(
        xs.astype(ml_dtypes.bfloat16)
        .reshape(B_PER_CORE, 8, 4, 128, C)
        .transpose(0, 1, 3, 2, 4)
        .reshape(B_PER_CORE, 8, 128, 4 * C)
    )
    # xt[b, pair, p, o, n] = fp8(x)[b, n, pair*256 + o*128 + p]
    xt = xs.transpose(0, 2, 1).astype(ml_dtypes.float8_e4m3)  # [2, C, HW]
    xt = np.ascontiguousarray(
        xt.reshape(B_PER_CORE, 2, 2, 128, HW)
        .transpose(0, 1, 3, 2, 4)
        .reshape(B_PER_CORE, 2, 128, 2 * HW)
    )
    return xb, xt


def run(x, beta, trace=False, **trace_kwargs):
    """Shard over batch, run on 8 cores, gather. Returns (out, BassKernelResults)."""
    x = np.asarray(x, dtype=np.float32)
    beta = np.asarray(beta, dtype=np.float32)
    assert x.shape == (B_FULL, H, W, C), x.shape
    nc = _build()
    xr = x.reshape(B_FULL, HW, C)
    in_maps = []
    for i in range(N_CORES):
        xb, xt = _prep_shard(xr, i)
        in_maps.append({"xb": xb, "xt": xt, "beta": beta})
    res = run_bass_kernel_spmd(
        nc, in_maps, core_ids=list(range(N_CORES)), trace=trace, **trace_kwargs
    )
    out = np.concatenate(
        [np.asarray(res.results[i]["out"]).astype(np.float32) for i in range(N_CORES)],
        axis=0,
    )
    return out.reshape(B_FULL, H, W, C), res


def kernel(x, beta):
    out, _ = run(x, beta, trace=False)
    return out


# revision 23
# speedup vs baseline: 1.0004x; 1.0004x over previous
"""Channel-attention kernel for Trainium2, data-parallel over batch on 8 NeuronCores.

Reference computation (per batch b):
    xr   = x[b].reshape(HW, C)                  # [4096, 512] fp32
    s    = xr^T @ xr                            # [C, C] gram matrix
    attn = softmax(s, axis=-1)
    v    = xr @ attn                            # [4096, 512]
    out  = beta * v + x[b]

Device strategy (per core: 2 batches, software-pipelined), v5:
  - the host shard step pre-packs three input views, each per-partition
    contiguous in DRAM so every load is a single-segment (cheap-trigger)
    DMA -- multi-segment triggers cost up to 14us of engine time:
      xb  = bf16 natural, half-batch packed   (epilogue, 2 loads/batch)\n    and the output is stored bf16 (the host upcasts to fp32): per-core DMA\n    sustains only ~300GB/s aggregate, so bytes are the wall -- bf16 stores\n    halve them at the same ~3e-4-per-element accuracy as the bf16 x load.
      xn  = fp8 natural, K-step packed        (GEMM1 operands, 4 loads/batch)
      xt  = fp8 x^T, channel-pair packed      (GEMM2 stationary, 2 loads/batch)
  - DMA-completion semaphore lanes are shared per tile-pool and waits use
    conservative emission-time thresholds, so each batch's input tiles live
    in their own pools (a later batch's in-flight loads must never gate an
    earlier batch's consumers).
  - both GEMMs run fp8 with perf_mode=DoubleRow (virtual 128x256 PE array).
    GEMM1 runs as two half-passes (cb{0,1} then cb{2,3}) so only 2 PSUM
    banks hold gram tiles and the first half's softmax hides under the
    second half's matmuls; each half ends cb-major for the same reason.
  - softmax rows on DVE+ScalarE out of PSUM; beta folded into the
    normalization (attn_scaled = beta * exp(s-max) / sum, written fp8), so
    beta=0 gives v==0 and out = fp32(bf16(x)) exactly.
  - GEMM2 into a 3-deep [128,1024] PSUM ring; the first chunks issue their
    channel-pair-0 matmuls only so the PE restarts before the last softmax
    finishes.  Epilogue (out = v + x): two of three chunks DVE-add straight
    from PSUM, every third is ScalarE-evicted to SBUF and GpSimd-added
    (GpSimd cannot read PSUM); 256-row chunks pair into 512-row stores
    alternating the sync/gpsimd queues.
"""

import ml_dtypes
import numpy as np

import concourse.bass as bass
import concourse.tile as tile
from concourse import bacc, mybir
from concourse.bass_utils import run_bass_kernel_spmd

N_CORES = 8
B_FULL = 16
B_PER_CORE = B_FULL // N_CORES  # 2
H = 64
W = 64
HW = H * W  # 4096
C = 512
NT = HW // 128  # 32 row tiles
CB = C // 128  # 4 channel blocks
NK = NT // 2  # 16 DoubleRow K-steps (256 rows each)
NKC = 4  # xn load chunks per batch (4 K-steps each)
NCH = NT // 2  # 16 GEMM2/epilogue chunks (256 rows each)
NSC = NT // 4  # 8 store super-chunks (512 rows each)
TAILK = 4  # K-steps per half-pass that run cb-major (softmax overlap)
# store spans (sc, ch0, ch1): big early stores amortize per-DMA cost, small
# final ones shorten the drain tail
STORE_SPANS = [(0, 0, 4), (1, 4, 8), (2, 8, 12), (3, 12, 14), (4, 14, 16)]
PREFIX = 3  # GEMM2 chunks that issue pair-0 matmuls before pair-1 is ready

F32 = mybir.dt.float32
BF16 = mybir.dt.bfloat16
FP8 = mybir.dt.float8e4
AXL = mybir.AxisListType
ALU = mybir.AluOpType
ACTFN = mybir.ActivationFunctionType
DR = mybir.MatmulPerfMode.DoubleRow


class BatchState:
    def __init__(self):
        self.xbf = {}  # eighth -> (tile, base_eighth) bf16 natural layout
        self.x8 = {}  # k-step -> [128, 2C] fp8 cast tiles (natural layout)
        self.xt8 = {}  # pair -> [128, 2*HW] fp8 tile (x^T, paired channels)
        self.s_ps = {}  # cb -> [128, C] f32 PSUM
        self.ot = {}  # store-span -> bf16 staging tile
        self.attn = {}  # pair -> [128, 2C] fp8 tile


def emit_xb_span(nc, pools, xb_ap, b, e0, e1, st, eng):
    """One DMA covering eighths [e0, e1) of batch b's bf16 natural view."""
    n = e1 - e0
    t = pools[f"xbf_{b}_{e0}"].tile(
        [128, n * 4 * C], BF16, tag="xbf", name=f"xbf_b{b}_e{e0}"
    )
    eng.dma_start(
        t[:, :].rearrange("p (e w) -> p e w", e=n),
        xb_ap[b, e0:e1, :, :].rearrange("e p w -> p e w"),
    )
    for e in range(e0, e1):
        st.xbf[e] = (t, e0)


def emit_all_loads(nc, pools, aps, states):
    """All loads up-front: aggregate DMA runs at its ~420GB/s cap, so what
    matters is per-queue deadline order and >=1MB DMAs (per-DMA queue cost
    ~2-3us).  Batch 0's bf16 view interleaves both queues in 1MB pairs so
    its casts are fed from ~12us; everything else follows by deadline."""
    xb_ap, xt_ap = aps
    emit_xb_span(nc, pools, xb_ap, 0, 0, 1, states[0], nc.sync)
    emit_xb_span(nc, pools, xb_ap, 0, 1, 2, states[0], nc.scalar)
    emit_xb_span(nc, pools, xb_ap, 0, 2, 4, states[0], nc.sync)
    emit_xb_span(nc, pools, xb_ap, 0, 4, 6, states[0], nc.scalar)
    emit_xb_span(nc, pools, xb_ap, 0, 6, 8, states[0], nc.sync)
    xt0 = pools["xt8_0"].tile([128, 4 * HW], FP8, tag="xt8", name="xt8_b0")
    nc.scalar.dma_start(
        xt0[:, :].rearrange("p (q w) -> p q w", q=2),
        xt_ap[0, :, :, :].rearrange("q p w -> p q w"),
    )
    states[0].xt8[0] = xt0
    emit_xb_span(nc, pools, xb_ap, 1, 0, 4, states[1], nc.sync)
    emit_xb_span(nc, pools, xb_ap, 1, 4, 8, states[1], nc.scalar)
    xt1 = pools["xt8_1"].tile([128, 4 * HW], FP8, tag="xt8", name="xt8_b1")
    nc.sync.dma_start(
        xt1[:, :].rearrange("p (q w) -> p q w", q=2),
        xt_ap[1, :, :, :].rearrange("q p w -> p q w"),
    )
    states[1].xt8[0] = xt1


def emit_cast(nc, pools, b, k, st, eng):
    """fp8 shadow of K-step k (row tiles 2k, 2k+1) on ScalarE or DVE."""
    e = (2 * k) // 4
    tile_, base = st.xbf[e]
    off = 2 * k - base * 4
    t = pools["x8"].tile([128, 2 * C], FP8, tag="x8", name=f"x8_b{b}_k{k}")
    src_slice = tile_[:, off * C : (off + 2) * C]
    if eng == "v":
        nc.vector.tensor_copy(t[:, :], src_slice)
    elif eng == "g":
        nc.gpsimd.tensor_copy(t[:, :], src_slice)
    else:
        nc.scalar.copy(t[:, :], src_slice)
    st.x8[k] = t


def emit_g1_step(nc, pools, b, k, cb, st):
    if k == 0:
        st.s_ps[cb] = pools["ps_s"].tile([128, C], F32, tag="s", name=f"s_b{b}_{cb}")
    r3 = st.x8[k][:, :].rearrange("p (o c) -> p o c", o=2)
    nc.tensor.matmul(
        st.s_ps[cb][:, :],
        r3[:, :, cb * 128 : (cb + 1) * 128],
        r3,
        start=(k == 0),
        stop=(k == NK - 1),
        perf_mode=DR,
    )


def emit_softmax(nc, pools, beta_bc, b, cb, st):
    """softmax row block cb out of PSUM -> fp8 half of the attn pair tile."""
    pair, o = cb // 2, cb % 2
    nmax = pools["st"].tile([128, 1], F32, tag="nmax")
    nc.vector.tensor_reduce(
        nmax[:, :], st.s_ps[cb][:, :], axis=AXL.X, op=ALU.max, negate=True
    )
    exps = pools["sm"].tile([128, C], BF16, tag="exps")
    ssum = pools["st"].tile([128, 1], F32, tag="ssum")
    nc.scalar.activation(
        exps[:, :],
        st.s_ps[cb][:, :],
        ACTFN.Exp,
        bias=nmax[:, :],
        scale=1.0,
        accum_out=ssum[:, :],
    )
    rinv = pools["st"].tile([128, 1], F32, tag="rinv")
    nc.vector.reciprocal(rinv[:, :], ssum[:, :])
    rsc = pools["st"].tile([128, 1], F32, tag="rsc")
    nc.vector.tensor_mul(rsc[:, :], rinv[:, :], beta_bc[:, :])
    if o == 0:
        at = pools["attn"].tile(
            [128, 2 * C], FP8, tag="attn", name=f"attn_b{b}_p{pair}"
        )
        st.attn[pair] = at
    nc.scalar.activation(
        st.attn[pair][:, o * C : (o + 1) * C], exps[:, :], ACTFN.Copy, scale=rsc[:, :]
    )


def emit_g1_half(
    nc, pools, beta_bc, b, half, st, cast_self=False, cast_next=None, after_k1=None
):
    """Half-pass over cb pair (2*half, 2*half+1); ends cb-major + softmax.
    cast_self: emit this batch's own fp8 casts alongside (pass A of batch 0).
    cast_next: BatchState of the next batch whose casts hide in this pass.
    after_k1: emitted once after K-step 1 (bulk load triggers go here so the
    first casts/matmuls aren't queued behind them)."""
    cbs = (2 * half, 2 * half + 1)
    for k in range(NK - TAILK):
        if cast_self:
            emit_cast(nc, pools, b, k, st, "s" if k % 2 == 0 else "v")
        for cb in cbs:
            emit_g1_step(nc, pools, b, k, cb, st)
        if k == 1 and after_k1 is not None:
            after_k1()
        if cast_next is not None:
            with nc._tile_ctx.tile_wait_until(0.038):
                emit_cast(nc, pools, b + 1, k, cast_next, "v" if k % 4 == 0 else "s")
    for k in range(NK - TAILK, NK):
        if cast_self:
            emit_cast(nc, pools, b, k, st, "s" if k % 2 == 0 else "v")
        if cast_next is not None:
            with nc._tile_ctx.tile_wait_until(0.038):
                emit_cast(nc, pools, b + 1, k, cast_next, "v" if k % 4 == 0 else "s")
    for cb in cbs:
        for k in range(NK - TAILK, NK):
            emit_g1_step(nc, pools, b, k, cb, st)
        emit_softmax(nc, pools, beta_bc, b, cb, st)


def emit_g2_mm(nc, b, ch, j, pair, st, vps):
    nt = ch * 2 + j
    xt3 = st.xt8[0][:, 2 * pair * HW : 2 * (pair + 1) * HW].rearrange(
        "p (o n) -> p o n", o=2
    )
    at3 = st.attn[pair][:, :].rearrange("p (o d) -> p o d", o=2)
    nc.tensor.matmul(
        vps[:, j * C : (j + 1) * C],
        xt3[:, :, nt * 128 : (nt + 1) * 128],
        at3,
        start=(pair == 0),
        stop=(pair == 1),
        perf_mode=DR,
    )


def emit_epilogue(nc, pools, out_ap, b, ch, st, vps):
    """out = v + x for chunk ch (row tiles 2ch, 2ch+1), then a 512KB store.
    Every fourth chunk takes the ScalarE-evict + GpSimd-add path to keep
    DVE off the critical path; stores alternate the sync/gpsimd queues."""
    sc, c0, c1 = next(x for x in STORE_SPANS if x[1] <= ch < x[2])
    if ch == c0:
        st.ot[sc] = pools["outp"].tile(
            [128, (c1 - c0) * 2 * C], BF16, tag=f"o{c1 - c0}",
            name=f"o_b{b}_s{sc}", bufs=3 if c1 - c0 == 4 else 2,
        )
    ot = st.ot[sc]
    oslice = ot[:, (ch - c0) * 2 * C : (ch - c0 + 1) * 2 * C]
    e = (2 * ch) // 4
    tile_, base = st.xbf[e]
    off = 2 * ch - base * 4
    xslice = tile_[:, off * C : (off + 2) * C]
    if ch % 4 == 1:
        tmp = pools["tmp"].tile([128, 2 * C], F32, tag="tmp")
        nc.scalar.copy(tmp[:, :], vps[:, :])
        nc.gpsimd.tensor_add(oslice, tmp[:, :], xslice)
    else:
        nc.vector.tensor_add(oslice, vps[:, :], xslice)
    if ch == c1 - 1:
        eng = nc.sync if sc % 2 == 0 else nc.scalar
        eng.dma_start(
            out_ap[b, c0 * 256 : c1 * 256, :].rearrange("(f p) c -> p f c", p=128),
            ot[:, :].rearrange("p (f c) -> p f c", c=C),
        )


def emit_g2(nc, pools, out_ap, b, st):
    vps_ring = {}
    for ch in range(PREFIX):
        vps_ring[ch] = pools["ps_v"].tile(
            [128, 2 * C], F32, tag="v", name=f"v_b{b}_c{ch}"
        )
        for j in range(2):
            emit_g2_mm(nc, b, ch, j, 0, st, vps_ring[ch])
    for ch in range(PREFIX):
        for j in range(2):
            emit_g2_mm(nc, b, ch, j, 1, st, vps_ring[ch])
        emit_epilogue(nc, pools, out_ap, b, ch, st, vps_ring[ch])
    for ch in range(PREFIX, NCH):
        vps = pools["ps_v"].tile([128, 2 * C], F32, tag="v", name=f"v_b{b}_c{ch}")
        for pair in range(2):
            for j in range(2):
                emit_g2_mm(nc, b, ch, j, pair, st, vps)
        emit_epilogue(nc, pools, out_ap, b, ch, st, vps)


def channel_attention_body(tc, out_ap, xb_ap, xt_ap, beta_ap):
    nc = tc.nc
    nc._tile_ctx = tc
    from contextlib import ExitStack

    with ExitStack() as ctx:
        ep = ctx.enter_context
        pools = {
            "attn": ep(tc.tile_pool(name="attn", bufs=4)),
            "sm": ep(tc.tile_pool(name="sm", bufs=3)),
            "st": ep(tc.tile_pool(name="st", bufs=8)),
            "outp": ep(tc.tile_pool(name="outp", bufs=3)),
            "tmp": ep(tc.tile_pool(name="tmp", bufs=3)),
            "const": ep(tc.tile_pool(name="const", bufs=1)),
            "ps_s": ep(tc.tile_pool(name="ps_s", bufs=2, space="PSUM")),
            "ps_v": ep(tc.tile_pool(name="ps_v", bufs=3, space="PSUM")),
        }
        pools["x8"] = ep(tc.tile_pool(name="x8", bufs=32))
        for b in range(B_PER_CORE):
            for e in (0, 1, 2, 4, 6) if b == 0 else (0, 4):
                pools[f"xbf_{b}_{e}"] = ep(tc.tile_pool(name=f"xbf_{b}_{e}", bufs=1))
        pools["xt8_0"] = ep(tc.tile_pool(name="xt8_0", bufs=1))
        pools["xt8_1"] = ep(tc.tile_pool(name="xt8_1", bufs=1))

        # beta -> broadcast to [128, 1] (gpsimd queue: keep sync/scalar clean)
        beta_sb = pools["const"].tile([1, 1], F32, tag="beta")
        nc.gpsimd.dma_start(beta_sb[0:1, 0:1], beta_ap[None, :])
        beta_bc = pools["const"].tile([128, 1], F32, tag="beta_bc")
        nc.gpsimd.partition_broadcast(beta_bc[:, :], beta_sb[0:1, :])

        aps = (xb_ap, xt_ap)
        states = [BatchState() for _ in range(B_PER_CORE)]
        emit_all_loads(nc, pools, aps, states)
        for b in range(B_PER_CORE):
            st = states[b]
            nxt = states[b + 1] if b + 1 < B_PER_CORE else None
            emit_g1_half(nc, pools, beta_bc, b, 0, st, cast_self=(b == 0))
            emit_g1_half(nc, pools, beta_bc, b, 1, st, cast_next=nxt)
            emit_g2(nc, pools, out_ap, b, st)


_NC_CACHE = None


def _build():
    global _NC_CACHE
    if _NC_CACHE is not None:
        return _NC_CACHE
    nc = bacc.Bacc(
        "TRN2",
        target_bir_lowering=False,
        debug=False,
        num_devices=N_CORES,
    )
    xb_ap = nc.dram_tensor(
        "xb", [B_PER_CORE, 8, 128, 4 * C], BF16, kind="ExternalInput"
    ).ap()
    xt_ap = nc.dram_tensor(
        "xt", [B_PER_CORE, 2, 128, 2 * HW], FP8, kind="ExternalInput"
    ).ap()
    beta_ap = nc.dram_tensor("beta", [1], F32, kind="ExternalInput").ap()
    out_ap = nc.dram_tensor(
        "out", [B_PER_CORE, HW, C], BF16, kind="ExternalOutput"
    ).ap()
    with tile.TileContext(nc) as tc:
        channel_attention_body(tc, out_ap, xb_ap, xt_ap, beta_ap)
    nc.compile()
    _NC_CACHE = nc
    return nc


def _prep_shard(xr, i):
    """Host-side input prep for core i: every view packed so each DMA is
    per-partition contiguous in DRAM."""
    xs = xr[i * B_PER_CORE : (i + 1) * B_PER_CORE]  # [2, HW, C] fp32
    # xb[b, q, p, f*C+c] = x[b, q*512 + f*128 + p, c]
    xb = np.ascontiguousarray(
        xs.astype(ml_dtypes.bfloat16)
        .reshape(B_PER_CORE, 8, 4, 128, C)
        .transpose(0, 1, 3, 2, 4)
        .reshape(B_PER_CORE, 8, 128, 4 * C)
    )
    # xt[b, pair, p, o, n] = fp8(x)[b, n, pair*256 + o*128 + p]
    xt = xs.transpose(0, 2, 1).astype(ml_dtypes.float8_e4m3)  # [2, C, HW]
    xt = np.ascontiguousarray(
        xt.reshape(B_PER_CORE, 2, 2, 128, HW)
        .transpose(0, 1, 3, 2, 4)
        .reshape(B_PER_CORE, 2, 128, 2 * HW)
    )
    return xb, xt


def run(x, beta, trace=False, **trace_kwargs):
    """Shard over batch, run on 8 cores, gather. Returns (out, BassKernelResults)."""
    x = np.asarray(x, dtype=np.float32)
    beta = np.asarray(beta, dtype=np.float32)
    assert x.shape == (B_FULL, H, W, C), x.shape
    nc = _build()
    xr = x.reshape(B_FULL, HW, C)
    in_maps = []
    for i in range(N_CORES):
        xb, xt = _prep_shard(xr, i)
        in_maps.append({"xb": xb, "xt": xt, "beta": beta})
    res = run_bass_kernel_spmd(
        nc, in_maps, core_ids=list(range(N_CORES)), trace=trace, **trace_kwargs
    )
    out = np.concatenate(
        [np.asarray(res.results[i]["out"]).astype(np.float32) for i in range(N_CORES)],
        axis=0,
    )
    return out.reshape(B_FULL, H, W, C), res


def kernel(x, beta):
    out, _ = run(x, beta, trace=False)
    return out


# revision 24
# speedup vs baseline: 1.0118x; 1.0114x over previous
"""Channel-attention kernel for Trainium2, data-parallel over batch on 8 NeuronCores.

Reference computation (per batch b):
    xr   = x[b].reshape(HW, C)                  # [4096, 512] fp32
    s    = xr^T @ xr                            # [C, C] gram matrix
    attn = softmax(s, axis=-1)
    v    = xr @ attn                            # [4096, 512]
    out  = beta * v + x[b]

Device strategy (per core: 2 batches, software-pipelined), v5:
  - the host shard step pre-packs three input views, each per-partition
    contiguous in DRAM so every load is a single-segment (cheap-trigger)
    DMA -- multi-segment triggers cost up to 14us of engine time:
      xb  = bf16 natural, half-batch packed   (epilogue, 2 loads/batch)\n    and the output is stored bf16 (the host upcasts to fp32): per-core DMA\n    sustains only ~300GB/s aggregate, so bytes are the wall -- bf16 stores\n    halve them at the same ~3e-4-per-element accuracy as the bf16 x load.
      xn  = fp8 natural, K-step packed        (GEMM1 operands, 4 loads/batch)
      xt  = fp8 x^T, channel-pair packed      (GEMM2 stationary, 2 loads/batch)
  - DMA-completion semaphore lanes are shared per tile-pool and waits use
    conservative emission-time thresholds, so each batch's input tiles live
    in their own pools (a later batch's in-flight loads must never gate an
    earlier batch's consumers).
  - both GEMMs run fp8 with perf_mode=DoubleRow (virtual 128x256 PE array).
    GEMM1 runs as two half-passes (cb{0,1} then cb{2,3}) so only 2 PSUM
    banks hold gram tiles and the first half's softmax hides under the
    second half's matmuls; each half ends cb-major for the same reason.
  - softmax rows on DVE+ScalarE out of PSUM; beta folded into the
    normalization (attn_scaled = beta * exp(s-max) / sum, written fp8), so
    beta=0 gives v==0 and out = fp32(bf16(x)) exactly.
  - GEMM2 into a 3-deep [128,1024] PSUM ring; the first chunks issue their
    channel-pair-0 matmuls only so the PE restarts before the last softmax
    finishes.  Epilogue (out = v + x): two of three chunks DVE-add straight
    from PSUM, every third is ScalarE-evicted to SBUF and GpSimd-added
    (GpSimd cannot read PSUM); 256-row chunks pair into 512-row stores
    alternating the sync/gpsimd queues.
"""

import ml_dtypes
import numpy as np

import concourse.bass as bass
import concourse.tile as tile
from concourse import bacc, mybir
from concourse.bass_utils import run_bass_kernel_spmd

N_CORES = 8
B_FULL = 16
B_PER_CORE = B_FULL // N_CORES  # 2
H = 64
W = 64
HW = H * W  # 4096
C = 512
NT = HW // 128  # 32 row tiles
CB = C // 128  # 4 channel blocks
NK = NT // 2  # 16 DoubleRow K-steps (256 rows each)
NKC = 4  # xn load chunks per batch (4 K-steps each)
NCH = NT // 2  # 16 GEMM2/epilogue chunks (256 rows each)
NSC = NT // 4  # 8 store super-chunks (512 rows each)
TAILK = 4  # K-steps per half-pass that run cb-major (softmax overlap)
# store spans (sc, ch0, ch1): big early stores amortize per-DMA cost, small
# final ones shorten the drain tail
STORE_SPANS = [(0, 0, 4), (1, 4, 8), (2, 8, 12), (3, 12, 14), (4, 14, 16)]
PREFIX = 3  # GEMM2 chunks that issue pair-0 matmuls before pair-1 is ready

F32 = mybir.dt.float32
BF16 = mybir.dt.bfloat16
FP8 = mybir.dt.float8e4
AXL = mybir.AxisListType
ALU = mybir.AluOpType
ACTFN = mybir.ActivationFunctionType
DR = mybir.MatmulPerfMode.DoubleRow


class BatchState:
    def __init__(self):
        self.xbf = {}  # eighth -> (tile, base_eighth) bf16 natural layout
        self.x8 = {}  # k-step -> [128, 2C] fp8 cast tiles (natural layout)
        self.xt8 = {}  # pair -> [128, 2*HW] fp8 tile (x^T, paired channels)
        self.s_ps = {}  # cb -> [128, C] f32 PSUM
        self.ot = {}  # store-span -> bf16 staging tile
        self.attn = {}  # pair -> [128, 2C] fp8 tile


def emit_xb_span(nc, pools, xb_ap, b, e0, e1, st, eng):
    """One DMA covering eighths [e0, e1) of batch b's bf16 natural view."""
    n = e1 - e0
    t = pools[f"xbf_{b}_{e0}"].tile(
        [128, n * 4 * C], BF16, tag="xbf", name=f"xbf_b{b}_e{e0}"
    )
    eng.dma_start(
        t[:, :].rearrange("p (e w) -> p e w", e=n),
        xb_ap[b, e0:e1, :, :].rearrange("e p w -> p e w"),
    )
    for e in range(e0, e1):
        st.xbf[e] = (t, e0)


def emit_all_loads(nc, pools, aps, states):
    """All loads up-front: aggregate DMA runs at its ~420GB/s cap, so what
    matters is per-queue deadline order and >=1MB DMAs (per-DMA queue cost
    ~2-3us).  Batch 0's bf16 view interleaves both queues in 1MB pairs so
    its casts are fed from ~12us; everything else follows by deadline."""
    xb_ap, xn_ap, xt_ap = aps

    def load_xn(b, h, eng):
        t = pools[f"xn8_{b}_{h}"].tile(
            [128, 16 * C], FP8, tag="xn8", name=f"xn8_b{b}_h{h}"
        )
        eng.dma_start(t[:, :], xn_ap[b, h, :, :])
        for kl in range(8):
            k = h * 8 + kl
            states[b].x8[k] = t[:, 2 * kl * C : 2 * (kl + 1) * C]

    def load_xt(b, eng):
        xt = pools[f"xt8_{b}"].tile([128, 4 * HW], FP8, tag="xt8", name=f"xt8_b{b}")
        eng.dma_start(
            xt[:, :].rearrange("p (q w) -> p q w", q=2),
            xt_ap[b, :, :, :].rearrange("q p w -> p q w"),
        )
        states[b].xt8[0] = xt

    load_xn(0, 0, nc.sync)
    load_xn(0, 1, nc.scalar)
    emit_xb_span(nc, pools, xb_ap, 0, 0, 4, states[0], nc.sync)
    load_xt(0, nc.scalar)
    load_xn(1, 0, nc.sync)
    emit_xb_span(nc, pools, xb_ap, 0, 4, 8, states[0], nc.scalar)
    load_xt(1, nc.sync)
    load_xn(1, 1, nc.scalar)
    emit_xb_span(nc, pools, xb_ap, 1, 0, 4, states[1], nc.sync)
    emit_xb_span(nc, pools, xb_ap, 1, 4, 8, states[1], nc.scalar)


def emit_g1_step(nc, pools, b, k, cb, st):
    if k == 0:
        st.s_ps[cb] = pools["ps_s"].tile([128, C], F32, tag="s", name=f"s_b{b}_{cb}")
    r3 = st.x8[k].rearrange("p (o c) -> p o c", o=2)
    nc.tensor.matmul(
        st.s_ps[cb][:, :],
        r3[:, :, cb * 128 : (cb + 1) * 128],
        r3,
        start=(k == 0),
        stop=(k == NK - 1),
        perf_mode=DR,
    )


def emit_softmax(nc, pools, beta_bc, b, cb, st):
    """softmax row block cb out of PSUM -> fp8 half of the attn pair tile."""
    pair, o = cb // 2, cb % 2
    nmax = pools["st"].tile([128, 1], F32, tag="nmax")
    nc.vector.tensor_reduce(
        nmax[:, :], st.s_ps[cb][:, :], axis=AXL.X, op=ALU.max, negate=True
    )
    exps = pools["sm"].tile([128, C], BF16, tag="exps")
    ssum = pools["st"].tile([128, 1], F32, tag="ssum")
    nc.scalar.activation(
        exps[:, :],
        st.s_ps[cb][:, :],
        ACTFN.Exp,
        bias=nmax[:, :],
        scale=1.0,
        accum_out=ssum[:, :],
    )
    rinv = pools["st"].tile([128, 1], F32, tag="rinv")
    nc.vector.reciprocal(rinv[:, :], ssum[:, :])
    rsc = pools["st"].tile([128, 1], F32, tag="rsc")
    nc.vector.tensor_mul(rsc[:, :], rinv[:, :], beta_bc[:, :])
    if o == 0:
        at = pools["attn"].tile(
            [128, 2 * C], FP8, tag="attn", name=f"attn_b{b}_p{pair}"
        )
        st.attn[pair] = at
    nc.scalar.activation(
        st.attn[pair][:, o * C : (o + 1) * C], exps[:, :], ACTFN.Copy, scale=rsc[:, :]
    )


def emit_g1_half(nc, pools, beta_bc, b, half, st):
    """Half-pass over cb pair (2*half, 2*half+1); ends cb-major + softmax."""
    cbs = (2 * half, 2 * half + 1)
    for k in range(NK - TAILK):
        for cb in cbs:
            emit_g1_step(nc, pools, b, k, cb, st)
    for cb in cbs:
        for k in range(NK - TAILK, NK):
            emit_g1_step(nc, pools, b, k, cb, st)
        emit_softmax(nc, pools, beta_bc, b, cb, st)


def emit_g2_mm(nc, b, ch, j, pair, st, vps):
    nt = ch * 2 + j
    xt3 = st.xt8[0][:, 2 * pair * HW : 2 * (pair + 1) * HW].rearrange(
        "p (o n) -> p o n", o=2
    )
    at3 = st.attn[pair][:, :].rearrange("p (o d) -> p o d", o=2)
    nc.tensor.matmul(
        vps[:, j * C : (j + 1) * C],
        xt3[:, :, nt * 128 : (nt + 1) * 128],
        at3,
        start=(pair == 0),
        stop=(pair == 1),
        perf_mode=DR,
    )


def emit_epilogue(nc, pools, out_ap, b, ch, st, vps):
    """out = v + x for chunk ch (row tiles 2ch, 2ch+1), then a 512KB store.
    Every fourth chunk takes the ScalarE-evict + GpSimd-add path to keep
    DVE off the critical path; stores alternate the sync/gpsimd queues."""
    sc, c0, c1 = next(x for x in STORE_SPANS if x[1] <= ch < x[2])
    if ch == c0:
        st.ot[sc] = pools["outp"].tile(
            [128, (c1 - c0) * 2 * C], BF16, tag=f"o{c1 - c0}",
            name=f"o_b{b}_s{sc}", bufs=3 if c1 - c0 == 4 else 2,
        )
    ot = st.ot[sc]
    oslice = ot[:, (ch - c0) * 2 * C : (ch - c0 + 1) * 2 * C]
    e = (2 * ch) // 4
    tile_, base = st.xbf[e]
    off = 2 * ch - base * 4
    xslice = tile_[:, off * C : (off + 2) * C]
    if ch % 4 == 1:
        tmp = pools["tmp"].tile([128, 2 * C], F32, tag="tmp")
        nc.scalar.copy(tmp[:, :], vps[:, :])
        nc.gpsimd.tensor_add(oslice, tmp[:, :], xslice)
    else:
        nc.vector.tensor_add(oslice, vps[:, :], xslice)
    if ch == c1 - 1:
        eng = nc.sync if sc % 2 == 0 else nc.scalar
        eng.dma_start(
            out_ap[b, c0 * 256 : c1 * 256, :].rearrange("(f p) c -> p f c", p=128),
            ot[:, :].rearrange("p (f c) -> p f c", c=C),
        )


def emit_g2(nc, pools, out_ap, b, st):
    vps_ring = {}
    for ch in range(PREFIX):
        vps_ring[ch] = pools["ps_v"].tile(
            [128, 2 * C], F32, tag="v", name=f"v_b{b}_c{ch}"
        )
        for j in range(2):
            emit_g2_mm(nc, b, ch, j, 0, st, vps_ring[ch])
    for ch in range(PREFIX):
        for j in range(2):
            emit_g2_mm(nc, b, ch, j, 1, st, vps_ring[ch])
        emit_epilogue(nc, pools, out_ap, b, ch, st, vps_ring[ch])
    for ch in range(PREFIX, NCH):
        vps = pools["ps_v"].tile([128, 2 * C], F32, tag="v", name=f"v_b{b}_c{ch}")
        for pair in range(2):
            for j in range(2):
                emit_g2_mm(nc, b, ch, j, pair, st, vps)
        emit_epilogue(nc, pools, out_ap, b, ch, st, vps)


def channel_attention_body(tc, out_ap, xb_ap, xn_ap, xt_ap, beta_ap):
    nc = tc.nc
    nc._tile_ctx = tc
    from contextlib import ExitStack

    with ExitStack() as ctx:
        ep = ctx.enter_context
        pools = {
            "attn": ep(tc.tile_pool(name="attn", bufs=4)),
            "sm": ep(tc.tile_pool(name="sm", bufs=3)),
            "st": ep(tc.tile_pool(name="st", bufs=8)),
            "outp": ep(tc.tile_pool(name="outp", bufs=3)),
            "tmp": ep(tc.tile_pool(name="tmp", bufs=3)),
            "const": ep(tc.tile_pool(name="const", bufs=1)),
            "ps_s": ep(tc.tile_pool(name="ps_s", bufs=2, space="PSUM")),
            "ps_v": ep(tc.tile_pool(name="ps_v", bufs=3, space="PSUM")),
        }
        for b in range(B_PER_CORE):
            for e in (0, 4):
                pools[f"xbf_{b}_{e}"] = ep(tc.tile_pool(name=f"xbf_{b}_{e}", bufs=1))
            for h in range(2):
                pools[f"xn8_{b}_{h}"] = ep(tc.tile_pool(name=f"xn8_{b}_{h}", bufs=1))
            pools[f"xt8_{b}"] = ep(tc.tile_pool(name=f"xt8_{b}", bufs=1))

        # beta -> broadcast to [128, 1] (gpsimd queue: keep sync/scalar clean)
        beta_sb = pools["const"].tile([1, 1], F32, tag="beta")
        nc.gpsimd.dma_start(beta_sb[0:1, 0:1], beta_ap[None, :])
        beta_bc = pools["const"].tile([128, 1], F32, tag="beta_bc")
        nc.gpsimd.partition_broadcast(beta_bc[:, :], beta_sb[0:1, :])

        aps = (xb_ap, xn_ap, xt_ap)
        states = [BatchState() for _ in range(B_PER_CORE)]
        emit_all_loads(nc, pools, aps, states)
        for b in range(B_PER_CORE):
            st = states[b]
            emit_g1_half(nc, pools, beta_bc, b, 0, st)
            emit_g1_half(nc, pools, beta_bc, b, 1, st)
            emit_g2(nc, pools, out_ap, b, st)


_NC_CACHE = None


def _build():
    global _NC_CACHE
    if _NC_CACHE is not None:
        return _NC_CACHE
    nc = bacc.Bacc(
        "TRN2",
        target_bir_lowering=False,
        debug=False,
        num_devices=N_CORES,
    )
    xb_ap = nc.dram_tensor(
        "xb", [B_PER_CORE, 8, 128, 4 * C], BF16, kind="ExternalInput"
    ).ap()
    xn_ap = nc.dram_tensor(
        "xn", [B_PER_CORE, 2, 128, 16 * C], FP8, kind="ExternalInput"
    ).ap()
    xt_ap = nc.dram_tensor(
        "xt", [B_PER_CORE, 2, 128, 2 * HW], FP8, kind="ExternalInput"
    ).ap()
    beta_ap = nc.dram_tensor("beta", [1], F32, kind="ExternalInput").ap()
    out_ap = nc.dram_tensor(
        "out", [B_PER_CORE, HW, C], BF16, kind="ExternalOutput"
    ).ap()
    with tile.TileContext(nc) as tc:
        channel_attention_body(tc, out_ap, xb_ap, xn_ap, xt_ap, beta_ap)
    nc.compile()
    _NC_CACHE = nc
    return nc


def _prep_shard(xr, i):
    """Host-side input prep for core i: every view packed so each DMA is
    per-partition contiguous in DRAM."""
    xs = xr[i * B_PER_CORE : (i + 1) * B_PER_CORE]  # [2, HW, C] fp32
    # xb[b, q, p, f*C+c] = x[b, q*512 + f*128 + p, c]
    xb = np.ascontiguousarray(
        xs.astype(ml_dtypes.bfloat16)
        .reshape(B_PER_CORE, 8, 4, 128, C)
        .transpose(0, 1, 3, 2, 4)
        .reshape(B_PER_CORE, 8, 128, 4 * C)
    )
    x8 = xs.astype(ml_dtypes.float8_e4m3)
    # xn[b, h, p, (kl o c)] = fp8(x)[b, (h*8+kl)*256 + o*128 + p, c]
    xn = np.ascontiguousarray(
        x8.reshape(B_PER_CORE, 2, 8, 2, 128, C)
        .transpose(0, 1, 4, 2, 3, 5)
        .reshape(B_PER_CORE, 2, 128, 16 * C)
    )
    # xt[b, pair, p, o, n] = fp8(x)[b, n, pair*256 + o*128 + p]
    xt = xs.transpose(0, 2, 1).astype(ml_dtypes.float8_e4m3)  # [2, C, HW]

    xt = np.ascontiguousarray(
        xt.reshape(B_PER_CORE, 2, 2, 128, HW)
        .transpose(0, 1, 3, 2, 4)
        .reshape(B_PER_CORE, 2, 128, 2 * HW)
    )
    return xb, xn, xt


def run(x, beta, trace=False, **trace_kwargs):
    """Shard over batch, run on 8 cores, gather. Returns (out, BassKernelResults)."""
    x = np.asarray(x, dtype=np.float32)
    beta = np.asarray(beta, dtype=np.float32)
    assert x.shape == (B_FULL, H, W, C), x.shape
    nc = _build()
    xr = x.reshape(B_FULL, HW, C)
    in_maps = []
    for i in range(N_CORES):
        xb, xn, xt = _prep_shard(xr, i)
        in_maps.append({"xb": xb, "xn": xn, "xt": xt, "beta": beta})
    res = run_bass_kernel_spmd(
        nc, in_maps, core_ids=list(range(N_CORES)), trace=trace, **trace_kwargs
    )
    out = np.concatenate(
        [np.asarray(res.results[i]["out"]).astype(np.float32) for i in range(N_CORES)],
        axis=0,
    )
    return out.reshape(B_FULL, H, W, C), res


def kernel(x, beta):
    out, _ = run(x, beta, trace=False)
    return out


# revision 25
# speedup vs baseline: 1.1015x; 1.0887x over previous
"""Channel-attention kernel for Trainium2, data-parallel over batch on 8 NeuronCores.

Reference computation (per batch b):
    xr   = x[b].reshape(HW, C)                  # [4096, 512] fp32
    s    = xr^T @ xr                            # [C, C] gram matrix
    attn = softmax(s, axis=-1)
    v    = xr @ attn                            # [4096, 512]
    out  = beta * v + x[b]

Device strategy (per core: 2 batches, software-pipelined), v5:
  - the host shard step pre-packs three input views, each per-partition
    contiguous in DRAM so every load is a single-segment (cheap-trigger)
    DMA -- multi-segment triggers cost up to 14us of engine time:
      xb  = bf16 natural, half-batch packed   (epilogue, 2 loads/batch)\n    and the output is stored bf16 (the host upcasts to fp32): per-core DMA\n    sustains only ~300GB/s aggregate, so bytes are the wall -- bf16 stores\n    halve them at the same ~3e-4-per-element accuracy as the bf16 x load.
      xn  = fp8 natural, K-step packed        (GEMM1 operands, 4 loads/batch)
      xt  = fp8 x^T, channel-pair packed      (GEMM2 stationary, 2 loads/batch)
  - DMA-completion semaphore lanes are shared per tile-pool and waits use
    conservative emission-time thresholds, so each batch's input tiles live
    in their own pools (a later batch's in-flight loads must never gate an
    earlier batch's consumers).
  - both GEMMs run fp8 with perf_mode=DoubleRow (virtual 128x256 PE array).
    GEMM1 runs as two half-passes (cb{0,1} then cb{2,3}) so only 2 PSUM
    banks hold gram tiles and the first half's softmax hides under the
    second half's matmuls; each half ends cb-major for the same reason.
  - softmax rows on DVE+ScalarE out of PSUM; beta folded into the
    normalization (attn_scaled = beta * exp(s-max) / sum, written fp8), so
    beta=0 gives v==0 and out = fp32(bf16(x)) exactly.
  - GEMM2 into a 3-deep [128,1024] PSUM ring; the first chunks issue their
    channel-pair-0 matmuls only so the PE restarts before the last softmax
    finishes.  Epilogue (out = v + x): two of three chunks DVE-add straight
    from PSUM, every third is ScalarE-evicted to SBUF and GpSimd-added
    (GpSimd cannot read PSUM); 256-row chunks pair into 512-row stores
    alternating the sync/gpsimd queues.
"""

import ml_dtypes
import numpy as np

import concourse.bass as bass
import concourse.tile as tile
from concourse import bacc, mybir
from concourse.bass_utils import run_bass_kernel_spmd

N_CORES = 8
B_FULL = 16
B_PER_CORE = B_FULL // N_CORES  # 2
H = 64
W = 64
HW = H * W  # 4096
C = 512
NT = HW // 128  # 32 row tiles
CB = C // 128  # 4 channel blocks
NK = NT // 2  # 16 DoubleRow K-steps (256 rows each)
NKC = 4  # xn load chunks per batch (4 K-steps each)
NCH = NT // 2  # 16 GEMM2/epilogue chunks (256 rows each)
NSC = NT // 4  # 8 store super-chunks (512 rows each)
TAILK = 4  # K-steps per half-pass that run cb-major (softmax overlap)
# store spans (sc, ch0, ch1): big early stores amortize per-DMA cost, small
# final ones shorten the drain tail
STORE_SPANS = [(0, 0, 4), (1, 4, 8), (2, 8, 12), (3, 12, 14), (4, 14, 16)]
PREFIX = 3  # GEMM2 chunks that issue pair-0 matmuls before pair-1 is ready

F32 = mybir.dt.float32
BF16 = mybir.dt.bfloat16
FP8 = mybir.dt.float8e4
AXL = mybir.AxisListType
ALU = mybir.AluOpType
ACTFN = mybir.ActivationFunctionType
DR = mybir.MatmulPerfMode.DoubleRow


class BatchState:
    def __init__(self):
        self.xbf = {}  # eighth -> (tile, base_eighth) bf16 natural layout
        self.x8 = {}  # k-step -> [128, 2C] fp8 cast tiles (natural layout)
        self.xt8 = {}  # pair -> [128, 2*HW] fp8 tile (x^T, paired channels)
        self.s_ps = {}  # cb -> [128, C] f32 PSUM
        self.ot = {}  # store-span -> bf16 staging tile
        self.attn = {}  # pair -> [128, 2C] fp8 tile


def emit_xb_span(nc, pools, xb_ap, b, e0, e1, st, eng):
    """One DMA covering eighths [e0, e1) of batch b's bf16 natural view."""
    n = e1 - e0
    t = pools[f"xbf_{b}_{e0}"].tile(
        [128, n * 4 * C], BF16, tag="xbf", name=f"xbf_b{b}_e{e0}"
    )
    eng.dma_start(
        t[:, :].rearrange("p (e w) -> p e w", e=n),
        xb_ap[b, e0:e1, :, :].rearrange("e p w -> p e w"),
    )
    for e in range(e0, e1):
        st.xbf[e] = (t, e0)


def emit_all_loads(nc, pools, aps, states):
    """All loads up-front: aggregate DMA runs at its ~420GB/s cap, so what
    matters is per-queue deadline order and >=1MB DMAs (per-DMA queue cost
    ~2-3us).  Batch 0's bf16 view interleaves both queues in 1MB pairs so
    its casts are fed from ~12us; everything else follows by deadline."""
    xb_ap, xn_ap, xt_ap = aps

    def load_xn(b, h, eng):
        t = pools[f"xn8_{b}_{h}"].tile(
            [128, 16 * C], FP8, tag="xn8", name=f"xn8_b{b}_h{h}"
        )
        eng.dma_start(t[:, :], xn_ap[b, h, :, :])
        for kl in range(8):
            k = h * 8 + kl
            states[b].x8[k] = t[:, 2 * kl * C : 2 * (kl + 1) * C]

    def load_xt(b, eng):
        xt = pools[f"xt8_{b}"].tile([128, 4 * HW], FP8, tag="xt8", name=f"xt8_b{b}")
        eng.dma_start(
            xt[:, :].rearrange("p (q w) -> p q w", q=2),
            xt_ap[b, :, :, :].rearrange("q p w -> p q w"),
        )
        states[b].xt8[0] = xt

    load_xn(0, 0, nc.sync)
    load_xn(0, 1, nc.scalar)
    load_xn(1, 1, nc.scalar)
    emit_xb_span(nc, pools, xb_ap, 0, 0, 4, states[0], nc.sync)
    load_xn(1, 0, nc.sync)
    load_xt(0, nc.scalar)
    load_xt(1, nc.sync)
    emit_xb_span(nc, pools, xb_ap, 0, 4, 8, states[0], nc.scalar)
    emit_xb_span(nc, pools, xb_ap, 1, 0, 4, states[1], nc.sync)
    emit_xb_span(nc, pools, xb_ap, 1, 4, 8, states[1], nc.scalar)


def emit_g1_step(nc, pools, b, k, cb, st):
    if k == 0:
        st.s_ps[cb] = pools["ps_s"].tile([128, C], F32, tag="s", name=f"s_b{b}_{cb}")
    r3 = st.x8[k].rearrange("p (o c) -> p o c", o=2)
    nc.tensor.matmul(
        st.s_ps[cb][:, :],
        r3[:, :, cb * 128 : (cb + 1) * 128],
        r3,
        start=(k == 0),
        stop=(k == NK - 1),
        perf_mode=DR,
    )


def emit_softmax(nc, pools, beta_bc, b, cb, st):
    """softmax row block cb out of PSUM -> fp8 half of the attn pair tile."""
    pair, o = cb // 2, cb % 2
    nmax = pools["st"].tile([128, 1], F32, tag="nmax")
    nc.vector.tensor_reduce(
        nmax[:, :], st.s_ps[cb][:, :], axis=AXL.X, op=ALU.max, negate=True
    )
    exps = pools["sm"].tile([128, C], BF16, tag="exps")
    ssum = pools["st"].tile([128, 1], F32, tag="ssum")
    nc.scalar.activation(
        exps[:, :],
        st.s_ps[cb][:, :],
        ACTFN.Exp,
        bias=nmax[:, :],
        scale=1.0,
        accum_out=ssum[:, :],
    )
    rinv = pools["st"].tile([128, 1], F32, tag="rinv")
    nc.vector.reciprocal(rinv[:, :], ssum[:, :])
    rsc = pools["st"].tile([128, 1], F32, tag="rsc")
    nc.vector.tensor_mul(rsc[:, :], rinv[:, :], beta_bc[:, :])
    if o == 0:
        at = pools["attn"].tile(
            [128, 2 * C], FP8, tag="attn", name=f"attn_b{b}_p{pair}"
        )
        st.attn[pair] = at
    nc.scalar.activation(
        st.attn[pair][:, o * C : (o + 1) * C], exps[:, :], ACTFN.Copy, scale=rsc[:, :]
    )


def emit_g1_half(nc, pools, beta_bc, b, half, st):
    """Half-pass over cb pair (2*half, 2*half+1); ends cb-major + softmax."""
    cbs = (2 * half, 2 * half + 1)
    for k in range(NK - TAILK):
        for cb in cbs:
            emit_g1_step(nc, pools, b, k, cb, st)
    for cb in cbs:
        for k in range(NK - TAILK, NK):
            emit_g1_step(nc, pools, b, k, cb, st)
        emit_softmax(nc, pools, beta_bc, b, cb, st)


def emit_g2_mm(nc, b, ch, j, pair, st, vps):
    nt = ch * 2 + j
    xt3 = st.xt8[0][:, 2 * pair * HW : 2 * (pair + 1) * HW].rearrange(
        "p (o n) -> p o n", o=2
    )
    at3 = st.attn[pair][:, :].rearrange("p (o d) -> p o d", o=2)
    nc.tensor.matmul(
        vps[:, j * C : (j + 1) * C],
        xt3[:, :, nt * 128 : (nt + 1) * 128],
        at3,
        start=(pair == 0),
        stop=(pair == 1),
        perf_mode=DR,
    )


def emit_epilogue(nc, pools, out_ap, b, ch, st, vps):
    """out = v + x for chunk ch (row tiles 2ch, 2ch+1), then a 512KB store.
    Every fourth chunk takes the ScalarE-evict + GpSimd-add path to keep
    DVE off the critical path; stores alternate the sync/gpsimd queues."""
    sc, c0, c1 = next(x for x in STORE_SPANS if x[1] <= ch < x[2])
    if ch == c0:
        st.ot[sc] = pools["outp"].tile(
            [128, (c1 - c0) * 2 * C], BF16, tag=f"o{c1 - c0}",
            name=f"o_b{b}_s{sc}", bufs=3 if c1 - c0 == 4 else 2,
        )
    ot = st.ot[sc]
    oslice = ot[:, (ch - c0) * 2 * C : (ch - c0 + 1) * 2 * C]
    e = (2 * ch) // 4
    tile_, base = st.xbf[e]
    off = 2 * ch - base * 4
    xslice = tile_[:, off * C : (off + 2) * C]
    if ch % 4 == 1:
        tmp = pools["tmp"].tile([128, 2 * C], F32, tag="tmp")
        nc.scalar.copy(tmp[:, :], vps[:, :])
        nc.gpsimd.tensor_add(oslice, tmp[:, :], xslice)
    else:
        nc.vector.tensor_add(oslice, vps[:, :], xslice)
    if ch == c1 - 1:
        eng = nc.sync if sc % 2 == 0 else nc.scalar
        eng.dma_start(
            out_ap[b, c0 * 256 : c1 * 256, :].rearrange("(f p) c -> p f c", p=128),
            ot[:, :].rearrange("p (f c) -> p f c", c=C),
        )


def emit_g2(nc, pools, out_ap, b, st):
    vps_ring = {}
    for ch in range(PREFIX):
        vps_ring[ch] = pools["ps_v"].tile(
            [128, 2 * C], F32, tag="v", name=f"v_b{b}_c{ch}"
        )
        for j in range(2):
            emit_g2_mm(nc, b, ch, j, 0, st, vps_ring[ch])
    for ch in range(PREFIX):
        for j in range(2):
            emit_g2_mm(nc, b, ch, j, 1, st, vps_ring[ch])
        emit_epilogue(nc, pools, out_ap, b, ch, st, vps_ring[ch])
    for ch in range(PREFIX, NCH):
        vps = pools["ps_v"].tile([128, 2 * C], F32, tag="v", name=f"v_b{b}_c{ch}")
        for pair in range(2):
            for j in range(2):
                emit_g2_mm(nc, b, ch, j, pair, st, vps)
        emit_epilogue(nc, pools, out_ap, b, ch, st, vps)


def channel_attention_body(tc, out_ap, xb_ap, xn_ap, xt_ap, beta_ap):
    nc = tc.nc
    nc._tile_ctx = tc
    from contextlib import ExitStack

    with ExitStack() as ctx:
        ep = ctx.enter_context
        pools = {
            "attn": ep(tc.tile_pool(name="attn", bufs=4)),
            "sm": ep(tc.tile_pool(name="sm", bufs=3)),
            "st": ep(tc.tile_pool(name="st", bufs=8)),
            "outp": ep(tc.tile_pool(name="outp", bufs=3)),
            "tmp": ep(tc.tile_pool(name="tmp", bufs=3)),
            "const": ep(tc.tile_pool(name="const", bufs=1)),
            "ps_s": ep(tc.tile_pool(name="ps_s", bufs=2, space="PSUM")),
            "ps_v": ep(tc.tile_pool(name="ps_v", bufs=3, space="PSUM")),
        }
        for b in range(B_PER_CORE):
            for e in (0, 4):
                pools[f"xbf_{b}_{e}"] = ep(tc.tile_pool(name=f"xbf_{b}_{e}", bufs=1))
            for h in range(2):
                pools[f"xn8_{b}_{h}"] = ep(tc.tile_pool(name=f"xn8_{b}_{h}", bufs=1))
            pools[f"xt8_{b}"] = ep(tc.tile_pool(name=f"xt8_{b}", bufs=1))

        # beta -> broadcast to [128, 1] (gpsimd queue: keep sync/scalar clean)
        beta_sb = pools["const"].tile([1, 1], F32, tag="beta")
        nc.gpsimd.dma_start(beta_sb[0:1, 0:1], beta_ap[None, :])
        beta_bc = pools["const"].tile([128, 1], F32, tag="beta_bc")
        nc.gpsimd.partition_broadcast(beta_bc[:, :], beta_sb[0:1, :])

        aps = (xb_ap, xn_ap, xt_ap)
        states = [BatchState() for _ in range(B_PER_CORE)]
        emit_all_loads(nc, pools, aps, states)
        for b in range(B_PER_CORE):
            st = states[b]
            emit_g1_half(nc, pools, beta_bc, b, 0, st)
            emit_g1_half(nc, pools, beta_bc, b, 1, st)
            emit_g2(nc, pools, out_ap, b, st)


_NC_CACHE = None


def _build():
    global _NC_CACHE
    if _NC_CACHE is not None:
        return _NC_CACHE
    nc = bacc.Bacc(
        "TRN2",
        target_bir_lowering=False,
        debug=False,
        num_devices=N_CORES,
    )
    xb_ap = nc.dram_tensor(
        "xb", [B_PER_CORE, 8, 128, 4 * C], BF16, kind="ExternalInput"
    ).ap()
    xn_ap = nc.dram_tensor(
        "xn", [B_PER_CORE, 2, 128, 16 * C], FP8, kind="ExternalInput"
    ).ap()
    xt_ap = nc.dram_tensor(
        "xt", [B_PER_CORE, 2, 128, 2 * HW], FP8, kind="ExternalInput"
    ).ap()
    beta_ap = nc.dram_tensor("beta", [1], F32, kind="ExternalInput").ap()
    out_ap = nc.dram_tensor(
        "out", [B_PER_CORE, HW, C], BF16, kind="ExternalOutput"
    ).ap()
    with tile.TileContext(nc) as tc:
        channel_attention_body(tc, out_ap, xb_ap, xn_ap, xt_ap, beta_ap)
    nc.compile()
    _NC_CACHE = nc
    return nc


def _prep_shard(xr, i):
    """Host-side input prep for core i: every view packed so each DMA is
    per-partition contiguous in DRAM."""
    xs = xr[i * B_PER_CORE : (i + 1) * B_PER_CORE]  # [2, HW, C] fp32
    # xb[b, q, p, f*C+c] = x[b, q*512 + f*128 + p, c]
    xb = np.ascontiguousarray(
        xs.astype(ml_dtypes.bfloat16)
        .reshape(B_PER_CORE, 8, 4, 128, C)
        .transpose(0, 1, 3, 2, 4)
        .reshape(B_PER_CORE, 8, 128, 4 * C)
    )
    x8 = xs.astype(ml_dtypes.float8_e4m3)
    # xn[b, h, p, (kl o c)] = fp8(x)[b, (h*8+kl)*256 + o*128 + p, c]
    xn = np.ascontiguousarray(
        x8.reshape(B_PER_CORE, 2, 8, 2, 128, C)
        .transpose(0, 1, 4, 2, 3, 5)
        .reshape(B_PER_CORE, 2, 128, 16 * C)
    )
    # xt[b, pair, p, o, n] = fp8(x)[b, n, pair*256 + o*128 + p]
    xt = xs.transpose(0, 2, 1).astype(ml_dtypes.float8_e4m3)  # [2, C, HW]

    xt = np.ascontiguousarray(
        xt.reshape(B_PER_CORE, 2, 2, 128, HW)
        .transpose(0, 1, 3, 2, 4)
        .reshape(B_PER_CORE, 2, 128, 2 * HW)
    )
    return xb, xn, xt


def run(x, beta, trace=False, **trace_kwargs):
    """Shard over batch, run on 8 cores, gather. Returns (out, BassKernelResults)."""
    x = np.asarray(x, dtype=np.float32)
    beta = np.asarray(beta, dtype=np.float32)
    assert x.shape == (B_FULL, H, W, C), x.shape
    nc = _build()
    xr = x.reshape(B_FULL, HW, C)
    in_maps = []
    for i in range(N_CORES):
        xb, xn, xt = _prep_shard(xr, i)
        in_maps.append({"xb": xb, "xn": xn, "xt": xt, "beta": beta})
    res = run_bass_kernel_spmd(
        nc, in_maps, core_ids=list(range(N_CORES)), trace=trace, **trace_kwargs
    )
    out = np.concatenate(
        [np.asarray(res.results[i]["out"]).astype(np.float32) for i in range(N_CORES)],
        axis=0,
    )
    return out.reshape(B_FULL, H, W, C), res


def kernel(x, beta):
    out, _ = run(x, beta, trace=False)
    return out
